# revision 28
# baseline (speedup 1.0000x reference)
"""Trainium2 Bass kernel for the ODLUE path-flow model (nn_AESUELOGIT).

Math (per reference):
  V[b,l]   = sum_f X[b,l,1+f]*theta[f] + theta_links[l]        (b = day*hour, 96)
  Vf[b,p]  = sum_l V[b,l]*D[l,p] + psc*log(psf[p])
  pf       = per-OD softmax over each OD's 4 consecutive paths
  f[b,p]   = pf * sqrt_q[od(p)]**2
  out[b,l] = relu(sum_p f[b,p]*D[l,p])

Distribution: shard the path axis P=20000 across 8 cores (2500 paths =
625 ODs per core; OD groups of 4 stay device-local). Each core computes
a partial link flow over its paths; host sums partials + relu.

Per-core dataflow (v2 — tuned to be HBM-streaming-bound end to end):
  The two big matmuls stream D / D^T from HBM exactly once, in fp8-e4m3
  (exact for D's 0/1 entries) with DoubleRow perf mode (K=256 per MM).
  V is split hi+lo into two fp8 matmuls accumulating in the same fp32
  PSUM bank (~bf16 accuracy at fp8 speed).  The DMA stream is hand-
  interleaved: X tiles first, then D chunks paced just ahead of the
  matmul1 consumption rate, with D^T chunks filling the spare bandwidth
  so matmul2 never starves at the end.
  V-chain runs in bf16 (2x DVE rate), theta_links folded into the first
  scalar_tensor_tensor op; the fp8-hi cast runs on the Scalar engine.
  softmax: exp on ACT straight out of PSUM (bf16 out; no max-subtract
  needed, |Vf| <~ 25), grouped sums of 4 on DVE, fast reciprocal,
  scale by q.
  matmul2: f^T tiles made by PE transposes batched 4-per-PSUM-bank with
  a single ACT copy per chunk to fp8; then a pure DR-matmul stream.
  The last K-pair is split per output chunk so the four link-flow
  chunks finish staggered, each copied (bf16) + DMA'd immediately.

Host prep (layout/sharding only): X channel 0 stripped and packed to
partition-major l-tiles [128, NLT, CH, B], D cast to fp8 and packed
[128, ktile, n] (plus transposed copy trimmed to 2000 links), per-core
slices of D/sqrt_q; host sums the 8 bf16 partials in f32 + relu.
"""

import sys
import types

import ml_dtypes
import numpy as np

# --- NTFF profile hook shim (missing antenv.axon_hooks in this image) ---
try:
    import antenv

    if "antenv.axon_hooks" not in sys.modules:
        _m = types.ModuleType("antenv.axon_hooks")
        _state = {}
        _m.set_axon_ntff_profile_hook = lambda h: _state.__setitem__("h", h)
        _m.get_axon_ntff_profile_hook = lambda: _state.get("h")
        sys.modules["antenv.axon_hooks"] = _m
        antenv.axon_hooks = _m
        try:
            from trn_agent_boot.trn_boot import _ntff_profile_via_ctypes

            _m.set_axon_ntff_profile_hook(
                _ntff_profile_via_ctypes("/opt/axon/libaxon_pjrt.so")
            )
        except Exception:
            pass
except Exception:
    pass

import concourse.bass as bass
import concourse.mybir as mybir
import concourse.tile as tile
from concourse import bacc
from concourse.bass import ds, ts
from concourse.bass_utils import run_bass_kernel_spmd
from concourse.masks import make_identity

BF = mybir.dt.bfloat16
F32 = mybir.dt.float32
FP8 = mybir.dt.float8e4
AF = mybir.ActivationFunctionType
ALU = mybir.AluOpType
AX = mybir.AxisListType
DR = mybir.MatmulPerfMode.DoubleRow

NCORES = 8
B = 96           # n_days * n_hours
L = 2000         # links
CH = 4           # X feature channels (ch 0 = tt_ff stripped on host)
P = 20000        # paths
PPG = 4          # paths per OD
PL = P // NCORES          # 2500 local paths
GL = PL // PPG            # 625 local ODs
NLT = 16                  # l-tiles (l padded to 2048 with zero D rows)
NPT = 20                  # p-tiles (p padded to 2560 with zero D^T rows)
PLP = NPT * 128           # 2560 padded local paths (pad: D cols 0, q 0)
GLP = PLP // PPG          # 640 padded local ODs
LP = NLT * 128            # 2048 padded links (K of matmul1 only)
NPC = 5                   # matmul1 psum chunks
PC = PLP // NPC           # 512 (exactly one PSUM bank)
PCG = PC // PPG           # 128 groups per chunk
NLC = 4                   # matmul2 psum chunks
LC = L // NLC             # 500 exact links per chunk (no pad in matmul2 N)

_CACHE = {}


def _build_nc(with_bias=True):
    key = ("nc", with_bias)
    if key in _CACHE:
        return _CACHE[key]
    nc = bacc.Bacc()

    xp_ext = nc.declare_dram_parameter("xp", [128, NLT, CH, B], BF, isOutput=False)
    # theta_links [:, :NLT] and theta [:, NLT:] packed into one per-partition
    # tensor: a single 128-element HWDGE transfer instead of slow SWDGE
    # broadcasts gating the V-chain
    thtl_ext = nc.declare_dram_parameter("thtl", [128, NLT + CH], BF, isOutput=False)
    d_ext = nc.declare_dram_parameter("dloc", [128, NLT, PLP], FP8, isOutput=False)
    dt_ext = nc.declare_dram_parameter("dtloc", [128, NPT, L], FP8, isOutput=False)
    sq_ext = nc.declare_dram_parameter("sq", [1, GLP], F32, isOutput=False)
    if with_bias:
        psf_ext = nc.declare_dram_parameter("psf", [1, PLP], F32, isOutput=False)
        psc_ext = nc.declare_dram_parameter("psc", [1, 1], F32, isOutput=False)
    out_ext = nc.declare_dram_parameter("out", [B, L], BF, isOutput=True)

    with tile.TileContext(nc) as tc:
        with (
            tc.tile_pool(name="const", bufs=1) as const,
            tc.tile_pool(name="work", bufs=1) as work,
        ):
            # ---- big loads on sync/HWDGE, strictly in consumption order ----
            # (HBM is the roofline: ~370 GB/s/core with all 8 cores active.
            # dsb must finish ~6 MB in before mm1's last k-pair; dtsb fills
            # the rest of the stream and lands just before mm2's tail.)
            # X groups (in l-tiles): first V tiles computed first.
            XGRP = [(0, 1), (1, 3), (4, 12)]
            xq_tiles = [
                work.tile([128, n, CH, B], BF, name=f"xq_{q}", tag=f"xq{q}")
                for q, (o, n) in enumerate(XGRP)
            ]
            dsb = work.tile([128, NLT, PLP], FP8)
            dtsb = work.tile([128, NPT, L], FP8)
            # zero the partition-trimmed last tiles BEFORE their (partial)
            # DMAs in program order, so the DMA data lands on top; on the
            # gpsimd queue nothing time-critical sits behind these now
            nc.gpsimd.memset(dsb[:, NLT - 1, :], 0.0)
            nc.gpsimd.memset(dtsb[:, NPT - 1, :], 0.0)

            # X and theta ride the scalar HWDGE ring: both rings ramp up in
            # parallel at t=0, so the sync ring's D stream isn't delayed by
            # the small head transfers (ring bring-up is ~5-8us each)
            def _xq(q):
                o, n = XGRP[q]
                nc.scalar.dma_start(out=xq_tiles[q], in_=xp_ext[:, o : o + n])

            def _dsb(g):
                if 2 * g + 2 == NLT:
                    nc.sync.dma_start(
                        out=dsb[:, 2 * g : 2 * g + 1],
                        in_=d_ext[:, 2 * g : 2 * g + 1],
                    )
                    nc.sync.dma_start(
                        out=dsb[: L % 128, 2 * g + 1 : 2 * g + 2],
                        in_=d_ext[: L % 128, 2 * g + 1 : 2 * g + 2],
                    )
                else:
                    nc.sync.dma_start(
                        out=dsb[:, 2 * g : 2 * g + 2],
                        in_=d_ext[:, 2 * g : 2 * g + 2],
                    )

            def _dtsb(j):
                if 2 * j + 2 == NPT:
                    nc.sync.dma_start(
                        out=dtsb[:, 2 * j : 2 * j + 1],
                        in_=dt_ext[:, 2 * j : 2 * j + 1],
                    )
                    nc.sync.dma_start(
                        out=dtsb[: PL % 128, 2 * j + 1 : 2 * j + 2],
                        in_=dt_ext[: PL % 128, 2 * j + 1 : 2 * j + 2],
                    )
                else:
                    nc.sync.dma_start(
                        out=dtsb[:, 2 * j : 2 * j + 2],
                        in_=dt_ext[:, 2 * j : 2 * j + 2],
                    )

            thtl_sb = const.tile([128, NLT + CH], BF)
            nc.scalar.dma_start(out=thtl_sb, in_=thtl_ext[:])
            _xq(0)
            _xq(1)
            _dsb(0)
            _dsb(1)
            _xq(2)
            for g in range(2, NLT // 2):
                _dsb(g)
            for j in range(NPT // 2):
                _dtsb(j)
            tl_sb = thtl_sb[:, :NLT]
            th_sb = thtl_sb[:, NLT:]

            # ---- small constants (SWDGE on gpsimd queue) ----
            sq_sb = const.tile([128, GLP], F32)
            nc.gpsimd.dma_start(out=sq_sb[:B], in_=sq_ext[:].to_broadcast([B, GLP]))
            if with_bias:
                psf_sb = const.tile([1, PLP], F32)
                nc.gpsimd.dma_start(out=psf_sb, in_=psf_ext[:])
                psc_sb = const.tile([1, 1], F32)
                nc.gpsimd.dma_start(out=psc_sb, in_=psc_ext[:])
                ones_sb = const.tile([1, B], BF)
                nc.vector.memset(ones_sb, 1.0)
            # warm the ACT Exp table early so softmax exp chunks don't pay
            # the ~1.4us table load mid-kernel
            dummy = const.tile([1, 8], F32)
            nc.vector.memset(dummy, 0.0)
            nc.scalar.activation(out=dummy, in_=dummy, func=AF.Exp)
            ident = const.tile([128, 128], BF)
            make_identity(nc, ident)
            # pre-warm the PE clock gate with dense real-shape matmuls; the
            # later ones are dep-chained to xq0 / the V-chain so PE activity
            # runs right up to the first real matmul (no re-gating)
            warm = const.tile([128, 512], BF)
            nc.vector.memset(warm, 0.0)
            pwcm = tc.tile_pool(name="pswarm", bufs=1, space="PSUM")
            pwp = pwcm.__enter__()
            pw = pwp.tile([128, 512], F32)
            for _ in range(6):
                nc.tensor.matmul(
                    pw[:96], lhsT=warm[:, :96], rhs=warm[:, :512],
                    start=True, stop=True,
                )
            for _ in range(3):
                nc.tensor.matmul(
                    pw[:96], lhsT=xq_tiles[0][:, 0, 0, :], rhs=warm[:, :512],
                    start=True, stop=True,
                )

            # qb = sqrt_q**2 broadcast over batch partitions (ACT, early)
            qb = const.tile([128, GLP], F32)
            nc.scalar.activation(out=qb[:B], in_=sq_sb[:B], func=AF.Square)

            # ---- V^T tiles (l on partitions), bf16 chain on DVE ----
            # vtf = X_c0*th0 + tl; vtf += X_c*th_c (c=1,2,3);
            # vthi = fp8(vtf) on ACT; vtlo = fp8(vtf - vthi) on DVE.
            vtf = work.tile([128, NLT, B], BF)
            vthi = work.tile([128, NLT, B], FP8)
            vtlo = work.tile([128, NLT, B], FP8)
            for g in range(len(XGRP)):
                o, n = XGRP[g]
                tsl = slice(o, o + n)
                xq = xq_tiles[g]
                tl_sl = tl_sb[:, tsl]
                tl_rep = bass.AP(
                    tensor=tl_sl.tensor,
                    offset=tl_sl.offset,
                    ap=[tl_sl.ap[0], tl_sl.ap[1], [0, B]],
                )
                nc.vector.scalar_tensor_tensor(
                    out=vtf[:, tsl], in0=xq[:, :, 0, :],
                    scalar=th_sb[:, 0:1],
                    in1=tl_rep, op0=ALU.mult, op1=ALU.add,
                )
                for c in (1, 2, 3):
                    nc.vector.scalar_tensor_tensor(
                        out=vtf[:, tsl], in0=xq[:, :, c, :],
                        scalar=th_sb[:, c : c + 1],
                        in1=vtf[:, tsl], op0=ALU.mult, op1=ALU.add,
                    )
                nc.scalar.copy(out=vthi[:, tsl], in_=vtf[:, tsl])
                nc.vector.tensor_sub(vtlo[:, tsl], vtf[:, tsl], vthi[:, tsl])
                if g == 0:
                    # last leg of the PE clock-ramp chain: depends on the
                    # first real weights, so it runs flush against matmul1
                    for _ in range(3):
                        nc.tensor.matmul(
                            pw[:96], lhsT=vthi[:, 0, :], rhs=dsb[:, 0, :512],
                            start=True, stop=True,
                        )

            if with_bias:
                # crow = psc * ln(psf)  (bf16 row, folded into matmul1 as K=1)
                lnp = const.tile([1, PLP], F32)
                nc.scalar.activation(out=lnp, in_=psf_sb, func=AF.Ln)
                crow = const.tile([1, PLP], BF)
                nc.vector.tensor_scalar_mul(crow, lnp, psc_sb[:, 0:1])

            e_sb = work.tile([128, PLP], BF)
            f_sb = work.tile([128, PLP], BF)
            s_sb = work.tile([128, GLP], F32)
            r_sb = work.tile([128, GLP], F32)
            t_sb = work.tile([128, GLP], BF)
            fT8 = work.tile([128, NPT, B], FP8)

            def _softmax_chunk(n, ps1):
                nc.scalar.activation(
                    out=e_sb[:B, ts(n, PC)], in_=ps1[n][:B], func=AF.Exp
                )
                e3 = e_sb[:B, ts(n, PC)].rearrange("p (g w) -> p g w", w=PPG)
                nc.vector.reduce_sum(
                    out=s_sb[:B, ds(n * PCG, PCG)], in_=e3, axis=AX.X
                )
                nc.vector.reciprocal_approx_fast(
                    out=r_sb[:B, ds(n * PCG, PCG)],
                    in_=s_sb[:B, ds(n * PCG, PCG)],
                )
                nc.vector.tensor_mul(
                    t_sb[:B, ds(n * PCG, PCG)],
                    r_sb[:B, ds(n * PCG, PCG)],
                    qb[:B, ds(n * PCG, PCG)],
                )
                t_sl = t_sb[:B, ds(n * PCG, PCG)]
                t_rep = bass.AP(
                    tensor=t_sl.tensor,
                    offset=t_sl.offset,
                    ap=[t_sl.ap[0], t_sl.ap[1], [0, PPG]],
                )
                f3 = f_sb[:B, ts(n, PC)].rearrange("p (g w) -> p g w", w=PPG)
                nc.vector.tensor_tensor(out=f3, in0=e3, in1=t_rep, op=ALU.mult)

            with tc.tile_pool(name="psT", bufs=2, space="PSUM") as psTp:

                def _transp_chunk(c):
                    # transpose chunk c's 4 p-tiles into one PSUM bank, then
                    # a single ACT copy to fp8
                    pT = psTp.tile([128, 4, B], BF)
                    for k in range(4):
                        j = 4 * c + k
                        nc.tensor.transpose(
                            pT[:, k, :], f_sb[:B, ds(128 * j, 128)], ident[:B, :B]
                        )
                    nc.scalar.copy(out=fT8[:, 4 * c : 4 * c + 4, :], in_=pT)

                ps1cm = tc.tile_pool(name="ps1", bufs=1, space="PSUM")
                ps1p = ps1cm.__enter__()
                ps1 = [
                    ps1p.tile([128, PC], F32, name=f"ps1_{n}", tag=f"b{n}")
                    for n in range(NPC)
                ]
                NG = NLT // 2
                GW = 5  # main-wave k-pairs; tail pairs 5..7 finish per chunk
                for g in range(GW):
                    gsl = slice(2 * g, 2 * g + 2)
                    for n in range(NPC):
                        nc.tensor.matmul(
                            ps1[n][:B],
                            lhsT=vthi[:, gsl, :],
                            rhs=dsb[:, gsl, ts(n, PC)],
                            start=(g == 0), stop=False, perf_mode=DR,
                        )
                        nc.tensor.matmul(
                            ps1[n][:B],
                            lhsT=vtlo[:, gsl, :],
                            rhs=dsb[:, gsl, ts(n, PC)],
                            start=False, stop=False, perf_mode=DR,
                        )
                # tail wave: chunk n fully accumulated ~1.3us after chunk
                # n-1, its softmax overlaps the next chunk's matmuls and its
                # transposes slot into the PE stream two chunks later
                for n in range(NPC):
                    for g in range(GW, NG):
                        gsl = slice(2 * g, 2 * g + 2)
                        nc.tensor.matmul(
                            ps1[n][:B],
                            lhsT=vthi[:, gsl, :],
                            rhs=dsb[:, gsl, ts(n, PC)],
                            start=False, stop=False, perf_mode=DR,
                        )
                        nc.tensor.matmul(
                            ps1[n][:B],
                            lhsT=vtlo[:, gsl, :],
                            rhs=dsb[:, gsl, ts(n, PC)],
                            start=False,
                            stop=(not with_bias and g == NG - 1),
                            perf_mode=DR,
                        )
                        if with_bias and g == NG - 1:
                            nc.tensor.matmul(
                                ps1[n][:B], lhsT=ones_sb[:1, :],
                                rhs=crow[:1, ts(n, PC)],
                                start=False, stop=True, skip_group_check=True,
                            )
                    _softmax_chunk(n, ps1)
                # transposes AFTER all mm1 matmuls: interleaving them pages
                # the in-order PE queue onto the serial DVE softmax chain
                for c in range(3):
                    _transp_chunk(c)
                ps1cm.__exit__(None, None, None)

                # ---- matmul2 interleaved with the remaining transposes so
                # the PE stream never idles: pairs 0..5 need only chunks
                # 0..2 of f^T; chunk 3/4 transposes slot in between ----
                out_sb = work.tile([128, L], BF)
                with tc.tile_pool(name="ps2", bufs=1, space="PSUM") as ps2p:
                    ps2 = [
                        ps2p.tile([128, LC], F32, name=f"ps2_{m}", tag=f"c{m}")
                        for m in range(NLC)
                    ]
                    NJ = NPT // 2

                    def _mm2_pair(j):
                        gsl = slice(2 * j, 2 * j + 2)
                        for m in range(NLC):
                            nc.tensor.matmul(
                                ps2[m][:B],
                                lhsT=fT8[:, gsl, :],
                                rhs=dtsb[:, gsl, ts(m, LC)],
                                start=(j == 0), stop=False, perf_mode=DR,
                            )

                    for j in range(6):
                        _mm2_pair(j)
                    _transp_chunk(3)
                    for j in range(6, 8):
                        _mm2_pair(j)
                    _transp_chunk(4)
                    _mm2_pair(8)
                    # last k-pair split per chunk: each chunk finishes, is
                    # copied (bf16) and DMA'd on the scalar HWDGE ring while
                    # later chunks still accumulate
                    gsl = slice(2 * (NJ - 1), 2 * NJ)
                    for m in range(NLC):
                        nc.tensor.matmul(
                            ps2[m][:B],
                            lhsT=fT8[:, gsl, :],
                            rhs=dtsb[:, gsl, ts(m, LC)],
                            start=False, stop=True, perf_mode=DR,
                        )
                        if m % 2 == 0:
                            nc.vector.tensor_copy(
                                out=out_sb[:B, ts(m, LC)], in_=ps2[m][:B]
                            )
                            nc.sync.dma_start(
                                out=out_ext[:, ts(m, LC)],
                                in_=out_sb[:B, ts(m, LC)],
                            )
                        else:
                            nc.scalar.copy(
                                out=out_sb[:B, ts(m, LC)], in_=ps2[m][:B]
                            )
                            nc.scalar.dma_start(
                                out=out_ext[:, ts(m, LC)],
                                in_=out_sb[:B, ts(m, LC)],
                            )
            pwcm.__exit__(None, None, None)

    nc.finalize()
    _CACHE[key] = nc
    return nc


def _prep_inputs(X, theta, theta_links, sqrt_q, psf, psc_factor, D):
    bf = ml_dtypes.bfloat16
    fp8 = ml_dtypes.float8_e4m3
    f32 = np.float32

    # X packed: xp[p, t, c, b] = X[b, 128t+p, 1+c], zero-padded l -> 2048
    Xf = np.asarray(X, f32).reshape(B, L, CH + 1)[:, :, 1:]  # [B, L, CH]
    Xf = Xf.transpose(1, 2, 0)                               # [L, CH, B]
    Xpad = np.zeros((NLT * 128, CH, B), f32)
    Xpad[:L] = Xf
    xp = np.ascontiguousarray(
        Xpad.reshape(NLT, 128, CH, B).transpose(1, 0, 2, 3)
    ).astype(bf)  # [128, NLT, CH, B]

    tlp = np.zeros((NLT * 128,), f32)
    tlp[:L] = np.asarray(theta_links, f32)
    tlp = np.ascontiguousarray(tlp.reshape(NLT, 128).T)  # [128, NLT]
    thtl = np.zeros((128, NLT + CH), f32)
    thtl[:, :NLT] = tlp
    thtl[:, NLT:] = np.asarray(theta, f32)[None, :]  # replicated per partition
    thtl = thtl.astype(bf)

    psc = np.asarray(psc_factor, f32).reshape(1, 1)
    D8 = np.asarray(D, f32).astype(fp8)  # exact for 0/1 entries

    in_maps = []
    for i in range(NCORES):
        pl = slice(i * PL, (i + 1) * PL)
        gl = slice(i * GL, (i + 1) * GL)
        sq_p = np.zeros((1, GLP), f32)
        sq_p[0, :GL] = np.asarray(sqrt_q, f32)[gl]
        dl = D8[:, pl]                                   # [2000, 2500]
        dpad = np.zeros((LP, PLP), fp8)
        dpad[:L, :PL] = dl
        dloc = np.ascontiguousarray(
            dpad.reshape(NLT, 128, PLP).transpose(1, 0, 2)
        )                                                # [128, NLT, PLP]
        dtpad = np.zeros((PLP, L), fp8)
        dtpad[:PL] = dl.T
        dtloc = np.ascontiguousarray(
            dtpad.reshape(NPT, 128, L).transpose(1, 0, 2)
        )                                                # [128, NPT, L]
        m = dict(xp=xp, thtl=thtl, dloc=dloc, dtloc=dtloc, sq=sq_p)
        if np.any(psc != 0.0):
            psf_p = np.ones((1, PLP), f32)
            psf_p[0, :PL] = np.asarray(psf, f32)[pl]
            m["psf"] = psf_p
            m["psc"] = psc
        in_maps.append(m)
    return in_maps


def run_on_cores(inputs, trace=False, **kw):
    """Compile (cached) + run SPMD on 8 cores; returns BassKernelResults."""
    with_bias = bool(np.any(np.asarray(inputs["psc_factor"], np.float32) != 0.0))
    nc = _build_nc(with_bias=with_bias)
    in_maps = _prep_inputs(
        inputs["X"], inputs["theta"], inputs["theta_links"], inputs["sqrt_q"],
        inputs["psf"], inputs["psc_factor"], inputs["D"],
    )
    return run_bass_kernel_spmd(
        nc, in_maps, core_ids=list(range(NCORES)), trace=trace, **kw
    )


def kernel(X, theta, theta_links, sqrt_q, psf, psc_factor, D, path_od=None):
    res = run_on_cores(
        dict(X=X, theta=theta, theta_links=theta_links, sqrt_q=sqrt_q,
             psf=psf, psc_factor=psc_factor, D=D)
    )
    acc = np.zeros((B, L), np.float32)
    for r in res.results:
        acc += np.asarray(r["out"], np.float32)
    return np.maximum(acc, 0.0).reshape(4, 24, L)
